# revision 10
# baseline (speedup 1.0000x reference)
"""KVCacheHeavyHitters eviction step as a Bass/Tile kernel on 8 TRN2 NeuronCores.

Head-parallel sharding: core c owns heads [4c, 4c+4). Per head:
  1. counts[l] = sum(attn_history[l, :] < 0)          (fused is_lt + accum on DVE)
  2. evict = argmax_l counts[l]  (first max)          (pack count*4096 + (4095-l), max)
  3. k/v/pos/cts bulk-copied input->output (pure DMA), then the evicted row is
     overwritten via indirect (scatter) DMA with the new k_val/v_val/input_pos/0.
"""

import sys

sys.path.insert(0, "/opt/trn_rl_repo")

import numpy as np

B, H, L, D, W = 1, 32, 4096, 128, 400
NCORES = 8
HPC = H // NCORES  # heads per core
P = 128
TPH = L // P  # row-tiles per head

_CACHE = {}


def _build_nc():
    import concourse.bass as bass
    import concourse.tile as tile
    from concourse import mybir

    Alu = mybir.AluOpType
    f32, i32 = mybir.dt.float32, mybir.dt.int32

    nc = bass.Bass(
        "TRN2",
        target_bir_lowering=False,
        debug=False,
        enable_asserts=True,
        num_devices=NCORES,
    )

    hist = nc.dram_tensor("hist", [HPC, L, W], f32, kind="ExternalInput")
    kc = nc.dram_tensor("kc", [HPC * L, D], f32, kind="ExternalInput")
    vc = nc.dram_tensor("vc", [HPC * L, D], f32, kind="ExternalInput")
    pos_in = nc.dram_tensor("pos_in", [HPC * L, 1], i32, kind="ExternalInput")
    cts_in = nc.dram_tensor("cts_in", [HPC * L, 1], i32, kind="ExternalInput")
    kval = nc.dram_tensor("kval", [HPC, D], f32, kind="ExternalInput")
    vval = nc.dram_tensor("vval", [HPC, D], f32, kind="ExternalInput")
    posval = nc.dram_tensor("posval", [HPC, 1], i32, kind="ExternalInput")
    iota_rev = nc.dram_tensor("iota_rev", [P, TPH], f32, kind="ExternalInput")
    head_base = nc.dram_tensor("head_base", [HPC, 1], i32, kind="ExternalInput")
    ident = nc.dram_tensor("ident", [P, P], f32, kind="ExternalInput")

    k_out = nc.dram_tensor("k_out", [HPC * L, D], f32, kind="ExternalOutput")
    v_out = nc.dram_tensor("v_out", [HPC * L, D], f32, kind="ExternalOutput")
    pos_out = nc.dram_tensor("pos_out", [HPC * L, 1], i32, kind="ExternalOutput")
    cts_out = nc.dram_tensor("cts_out", [HPC * L, 1], i32, kind="ExternalOutput")

    from concourse.tile_rust import add_dep_helper

    with tile.TileContext(nc) as tc:
        with (
            tc.tile_pool(name="histp", bufs=8) as hist_pool,
            tc.tile_pool(name="scratch", bufs=4) as scratch_pool,
            tc.tile_pool(name="small", bufs=1) as small_pool,
            tc.tile_pool(name="psum", bufs=1, space="PSUM") as psum_pool,
        ):
            all_dmas = []
            iota_sb = small_pool.tile([P, TPH], f32)
            all_dmas.append(nc.sync.dma_start(out=iota_sb[:], in_=iota_rev[:, :]))
            ident_sb = small_pool.tile([P, P], f32)
            all_dmas.append(nc.sync.dma_start(out=ident_sb[:], in_=ident[:, :]))
            hb_sb = small_pool.tile([HPC, 1], i32)
            all_dmas.append(nc.sync.dma_start(out=hb_sb[:], in_=head_base[:, :]))
            kval_sb = small_pool.tile([HPC, D], f32)
            all_dmas.append(nc.sync.dma_start(out=kval_sb[:], in_=kval[:, :]))
            vval_sb = small_pool.tile([HPC, D], f32)
            all_dmas.append(nc.sync.dma_start(out=vval_sb[:], in_=vval[:, :]))
            posval_sb = small_pool.tile([HPC, 1], i32)
            all_dmas.append(nc.sync.dma_start(out=posval_sb[:], in_=posval[:, :]))
            zero_sb = small_pool.tile([HPC, 1], i32)
            nc.vector.memset(zero_sb[:], 0)

            # Phase 1: negative-entry counts per cache slot.
            # 2-D loads (plain DMA struct), compares into a grouped scratch,
            # one multi-column reduce per G tiles.
            G = 8
            counts = small_pool.tile([P, HPC * TPH], f32)
            for h in range(HPC):
                for jg in range(TPH // G):
                    s = scratch_pool.tile([P, G, W], f32)
                    for g in range(G):
                        j = jg * G + g
                        n_dma = len(all_dmas)
                        gate = None
                        if n_dma >= 8:
                            # The DMAHW-lane sems round-robin over 8 lanes; a
                            # DMA's lane wait targets the DMA 8 issues back.
                            # Absorb that wait into an SP nop so the DMA's
                            # single ISA wait slot is free for its data dep.
                            gate = nc.sync.nop(hint="lanegate")
                            add_dep_helper(
                                gate.ins, all_dmas[n_dma - 8].ins, sync=True,
                                reason="lane gate",
                            )
                        t = hist_pool.tile([P, W], f32)
                        dma = nc.sync.dma_start(
                            out=t[:], in_=hist[h, j * P : (j + 1) * P, :]
                        )
                        if gate is not None:
                            add_dep_helper(
                                dma.ins, gate.ins, sync=False,
                                reason="dma after lane gate",
                            )
                        all_dmas.append(dma)
                        nc.vector.tensor_scalar(
                            out=s[:, g, :],
                            in0=t[:],
                            scalar1=0.0,
                            scalar2=None,
                            op0=Alu.is_lt,
                        )
                    base = h * TPH + jg * G
                    nc.vector.tensor_reduce(
                        out=counts[:, base : base + G],
                        in_=s[:],
                        axis=mybir.AxisListType.X,
                        op=Alu.add,
                    )

            # Phase 2: packed argmax. packed = count*4096 + (4095 - slot)
            packed = small_pool.tile([P, HPC * TPH], f32)
            pmax = small_pool.tile([P, HPC], f32)
            for h in range(HPC):
                blk = slice(h * TPH, (h + 1) * TPH)
                nc.vector.scalar_tensor_tensor(
                    out=packed[:, blk],
                    in0=counts[:, blk],
                    scalar=float(L),
                    in1=iota_sb[:],
                    op0=Alu.mult,
                    op1=Alu.add,
                )
                nc.vector.reduce_max(
                    out=pmax[:, h : h + 1],
                    in_=packed[:, blk],
                    axis=mybir.AxisListType.X,
                )

            pm_t = psum_pool.tile([P, P], f32, space="PSUM")
            nc.tensor.transpose(out=pm_t[:HPC, :], in_=pmax[:], identity=ident_sb[:])

            gmax = small_pool.tile([HPC, 1], f32)
            nc.vector.reduce_max(
                out=gmax[:], in_=pm_t[:HPC, :], axis=mybir.AxisListType.X
            )
            gmax_i = small_pool.tile([HPC, 1], i32)
            nc.vector.tensor_copy(out=gmax_i[:], in_=gmax[:])
            rem = small_pool.tile([HPC, 1], i32)
            nc.vector.tensor_scalar(
                out=rem[:], in0=gmax_i[:], scalar1=L - 1, scalar2=None,
                op0=Alu.bitwise_and,
            )
            evict = small_pool.tile([HPC, 1], i32)
            nc.vector.tensor_scalar(
                out=evict[:], in0=rem[:], scalar1=L - 1, scalar2=None,
                op0=Alu.bitwise_xor,
            )
            gidx = small_pool.tile([HPC, 1], i32)
            nc.vector.tensor_tensor(
                out=gidx[:], in0=evict[:], in1=hb_sb[:], op=Alu.add
            )

            # Phase 3: bulk passthrough copies (DRAM->DRAM).
            CH = 4096  # rows per chunk (2 MiB)

            def gated_dma(out_ap, in_ap):
                n_dma = len(all_dmas)
                gate = None
                if n_dma >= 8:
                    gate = nc.sync.nop(hint="lanegate")
                    add_dep_helper(
                        gate.ins, all_dmas[n_dma - 8].ins, sync=True,
                        reason="lane gate",
                    )
                d = nc.sync.dma_start(out=out_ap, in_=in_ap)
                if gate is not None:
                    add_dep_helper(d.ins, gate.ins, sync=False, reason="after gate")
                all_dmas.append(d)
                return d

            k_copies, v_copies = [], []
            for r in range(0, HPC * L, CH):
                k_copies.append(gated_dma(k_out[r : r + CH, :], kc[r : r + CH, :]))
                v_copies.append(gated_dma(v_out[r : r + CH, :], vc[r : r + CH, :]))
            pos_copy = gated_dma(pos_out[:, :], pos_in[:, :])
            cts_copy = gated_dma(cts_out[:, :], cts_in[:, :])

            # Phase 4: scatter the evicted rows (after the bulk copies land).
            sc_k = nc.gpsimd.indirect_dma_start(
                out=k_out[:, :],
                out_offset=bass.IndirectOffsetOnAxis(ap=gidx[:, :1], axis=0),
                in_=kval_sb[:],
                in_offset=None,
            )
            sc_v = nc.gpsimd.indirect_dma_start(
                out=v_out[:, :],
                out_offset=bass.IndirectOffsetOnAxis(ap=gidx[:, :1], axis=0),
                in_=vval_sb[:],
                in_offset=None,
            )
            sc_pos = nc.gpsimd.indirect_dma_start(
                out=pos_out[:, :],
                out_offset=bass.IndirectOffsetOnAxis(ap=gidx[:, :1], axis=0),
                in_=posval_sb[:],
                in_offset=None,
            )
            sc_cts = nc.gpsimd.indirect_dma_start(
                out=cts_out[:, :],
                out_offset=bass.IndirectOffsetOnAxis(ap=gidx[:, :1], axis=0),
                in_=zero_sb[:],
                in_offset=None,
            )
            # One gpsimd nop per bulk copy (each nop carries a single sem
            # wait), chained in order; all scatters order after the chain.
            prev = None
            for cp in k_copies + v_copies + [pos_copy, cts_copy]:
                n = nc.gpsimd.nop(hint="copygate")
                add_dep_helper(n.ins, cp.ins, sync=True, reason="await bulk copy")
                if prev is not None:
                    add_dep_helper(n.ins, prev.ins, sync=False, reason="chain")
                prev = n
            for sc in (sc_k, sc_v, sc_pos, sc_cts):
                add_dep_helper(sc.ins, prev.ins, sync=False, reason="scatter after copies")

    _split_multiwaits(nc, mybir)
    return nc


def _split_multiwaits(nc, mybir):
    """walrus allows ~2 sync commands per instruction (waits + updates).

    Hoist all-but-the-last wait of any multi-wait instruction onto fresh
    same-engine nops placed immediately before it — waits execute earlier on
    the same engine, which is strictly conservative.
    """
    # pass 1: find offenders
    offenders = []  # (bb, ins)
    for func in nc.m.functions:
        for bb in func.blocks:
            for ins in bb.instructions:
                si = ins.sync_info
                if si and si.on_wait and len(si.on_wait) > 1:
                    offenders.append((bb, ins))
    if not offenders:
        return
    # pass 2: create nops (they are appended to the trailing bb); give each
    # one hoisted wait, then pull them out of wherever they landed.
    plan = {}  # ins.name -> list of nop instructions to place before it
    all_nops = set()
    for bb, ins in offenders:
        waits = list(ins.sync_info.on_wait)
        nops = []
        for w in waits[:-1]:
            n = nc.engines[ins.engine].nop(hint="wsplit")
            n.ins.sync_info = mybir.SyncInfo(on_wait=[w], on_update=[])
            nops.append(n.ins)
            all_nops.add(n.ins.name)
        ins.sync_info.on_wait = [waits[-1]]
        plan[ins.name] = nops
    # pass 3: rebuild instruction lists — drop stray nop appends, insert
    # each nop right before its target.
    for func in nc.m.functions:
        for bb in func.blocks:
            cur = list(bb.instructions)
            rebuilt = []
            for ins in cur:
                if ins.name in all_nops:
                    continue
                rebuilt.extend(plan.get(ins.name, ()))
                rebuilt.append(ins)
            if len(rebuilt) != len(cur):
                live = bb.instructions
                while len(live):
                    live.pop()
                live.extend(rebuilt)


def _get_nc():
    if "nc" not in _CACHE:
        _CACHE["nc"] = _build_nc()
    return _CACHE["nc"]


def _make_in_maps(inputs):
    attn_history = np.asarray(inputs["attn_history"], dtype=np.float32)
    k_cache = np.asarray(inputs["k_cache"], dtype=np.float32)
    v_cache = np.asarray(inputs["v_cache"], dtype=np.float32)
    pos = np.asarray(inputs["pos"], dtype=np.int32)
    attn_cts = np.asarray(inputs["attn_cts"], dtype=np.int32)
    k_val = np.asarray(inputs["k_val"], dtype=np.float32)
    v_val = np.asarray(inputs["v_val"], dtype=np.float32)
    input_pos = np.asarray(inputs["input_pos"], dtype=np.int32)

    # constants
    gi = np.arange(L, dtype=np.float32).reshape(TPH, P).T  # [p, j] = j*128 + p
    iota_rev = (L - 1) - gi
    head_base = (np.arange(HPC, dtype=np.int32) * L).reshape(HPC, 1)
    ident = np.eye(P, dtype=np.float32)

    in_maps = []
    for c in range(NCORES):
        hs = slice(c * HPC, (c + 1) * HPC)
        in_maps.append(
            {
                "hist": np.ascontiguousarray(attn_history[0, hs]),
                "kc": np.ascontiguousarray(k_cache[0, hs]).reshape(HPC * L, D),
                "vc": np.ascontiguousarray(v_cache[0, hs]).reshape(HPC * L, D),
                "pos_in": np.ascontiguousarray(pos[0, hs]).reshape(HPC * L, 1),
                "cts_in": np.ascontiguousarray(attn_cts[0, hs]).reshape(HPC * L, 1),
                "kval": np.ascontiguousarray(k_val[0, hs, 0]),
                "vval": np.ascontiguousarray(v_val[0, hs, 0]),
                "posval": np.full((HPC, 1), input_pos[0], dtype=np.int32),
                "iota_rev": iota_rev,
                "head_base": head_base,
                "ident": ident,
            }
        )
    return in_maps


def _assemble(results):
    k_new = np.concatenate(
        [results[c]["k_out"].reshape(1, HPC, L, D) for c in range(NCORES)], axis=1
    )
    v_new = np.concatenate(
        [results[c]["v_out"].reshape(1, HPC, L, D) for c in range(NCORES)], axis=1
    )
    pos_new = np.concatenate(
        [results[c]["pos_out"].reshape(1, HPC, L) for c in range(NCORES)], axis=1
    )
    cts_new = np.concatenate(
        [results[c]["cts_out"].reshape(1, HPC, L) for c in range(NCORES)], axis=1
    )
    kv = np.stack([k_new, v_new], axis=0)
    return kv, pos_new.astype(np.int32), cts_new.astype(np.int32)


def _get_profile_hook():
    """The agent image lacks antenv.axon_hooks; build the ctypes NTFF hook
    directly from the boot helper."""
    from trn_agent_boot.trn_boot import _ntff_profile_via_ctypes

    return _ntff_profile_via_ctypes("/opt/axon/libaxon_pjrt.so")


def _exec_time_from_ntff(outdir):
    """neuron-profile view each NTFF; return max span (ns) across cores."""
    import glob
    import json
    import subprocess

    ntffs = sorted(glob.glob(outdir + "/*.ntff"))
    if not ntffs:
        print(f"no NTFFs in {outdir}: {sorted(__import__('os').listdir(outdir))}")
        return None
    neffs = sorted(
        glob.glob(outdir + "/*.neff"),
        key=lambda p: __import__("os").path.getsize(p),
    )
    if not neffs:
        import os

        cache = glob.glob("/root/.neuron-compile-cache/**/*.neff", recursive=True)
        neffs = sorted(cache, key=os.path.getmtime)
    if not neffs:
        print("no NEFF found for profile view")
        return None
    neff = neffs[-1]
    best = None
    for i, ntff in enumerate(ntffs):
        jf = f"{outdir}/prof_{i}.json"
        cmd = [
            "neuron-profile",
            "view",
            "--ignore-nc-buf-usage",
            "-s",
            ntff,
            "-n",
            neff,
            "--output-format=json",
            f"--output-file={jf}",
        ]
        try:
            subprocess.check_call(cmd, cwd=outdir)
        except subprocess.CalledProcessError as e:
            print(f"neuron-profile failed on {ntff}: {e}")
            continue
        with open(jf) as f:
            data = json.load(f)
        lo, hi = None, None
        for section in ("instruction", "dma"):
            for ev in data.get(section) or []:
                ts = ev.get("timestamp")
                dur = ev.get("duration")
                if ts is None:
                    continue
                dur = dur or 0
                lo = ts if lo is None else min(lo, ts)
                hi = ts + dur if hi is None else max(hi, ts + dur)
        if lo is not None:
            span = hi - lo
            best = span if best is None else max(best, span)
    return best


def run(trace=False, **inputs):
    """Run on the 8 NeuronCores; returns ((kv, pos, cts), exec_time_ns)."""
    from concourse import bass2jax

    nc = _get_nc()
    in_maps = _make_in_maps(inputs)
    if not trace:
        results = bass2jax.run_bass_via_pjrt(nc, in_maps, NCORES)
        return _assemble(results), None

    import tempfile

    outdir = tempfile.mkdtemp(prefix="ntff_")
    hook = _get_profile_hook()
    with hook(outdir, [0]):
        results = bass2jax.run_bass_via_pjrt(nc, in_maps, NCORES)
    exec_ns = _exec_time_from_ntff(outdir)
    print(f"profile dir: {outdir}")
    return _assemble(results), exec_ns


def kernel(**inputs):
    out, _ = run(trace=False, **inputs)
    return out
